# revision 2
# baseline (speedup 1.0000x reference)
"""Bahdanau additive attention on 8 TRN2 NeuronCores (Bass/Tile).

Reference computation (B=4, T=512, S=512, D=256, IN=512):
    wq[b,t,d]   = sum_i x[b,t,i]   * Wq[d,i]
    uh[b,s,d]   = sum_m mems[b,s,m]* Wc[d,m] + bc[d]
    align[b,t,s]= sum_d v[d] * tanh(wq[b,t,d] + uh[b,s,d])     (masked s>=L_b -> -inf)
    av          = softmax_s(align)
    c[b,t,m]    = sum_s av[b,t,s] * mems[b,s,m]
    attn[b,t,o] = sum_k [c|x][b,t,k] * Wout[o,k] + bout[o]
    returns (attn, av)

Sharding: 16 (batch, t-tile-of-128) blocks, 2 per core (pure data parallel,
no collectives).  Per (t, d-half): DVE broadcast-add z = uh + wq[t] (fp32),
ACT tanh batched over 8 such slices (one big ACTIVATE), PE reduces over the
d-partition dim with a 32-column one-hot v weight into the PSUM row for t.
Softmax is exp -> multiplicative mask -> sum -> reciprocal scale (no max
subtraction needed: |align| <= sum|v| ~ 10).  All matmul inputs bf16,
accumulation fp32.
"""
import numpy as np
import ml_dtypes
from contextlib import ExitStack

import concourse.bass as bass
import concourse.bacc as bacc
import concourse.mybir as mybir
import concourse.tile as tile
from concourse.bass_utils import run_bass_kernel_spmd

F32 = mybir.dt.float32
BF16 = mybir.dt.bfloat16
TANH = mybir.ActivationFunctionType.Tanh
EXP = mybir.ActivationFunctionType.Exp
BF = ml_dtypes.bfloat16

B, T, S, D, IN = 4, 512, 512, 256, 512
NC = 8           # cores
NJ = 2           # t-tiles per core
TT = 128         # t rows per tile
G = 4            # t's per ACT batch (free = G*2*S = 4096)

_BUILT = None
LAST_RESULT = None


def _build():
    nc = bacc.Bacc("TRN2", target_bir_lowering=False, debug=False,
                   enable_asserts=False, num_devices=NC)

    xT_d = nc.dram_tensor("xT", [NJ, IN, TT], BF16, kind="ExternalInput")
    memsT_d = nc.dram_tensor("memsT", [NJ, D, S], BF16, kind="ExternalInput")
    memsL_d = nc.dram_tensor("memsL", [NJ, S, D], BF16, kind="ExternalInput")
    mask_d = nc.dram_tensor("mask", [NJ, 128, S], F32, kind="ExternalInput")
    WqT_d = nc.dram_tensor("WqT", [IN, D], BF16, kind="ExternalInput")
    WcT_d = nc.dram_tensor("WcT", [D, D], BF16, kind="ExternalInput")
    vcols_d = nc.dram_tensor("vcols", [128, 2, 32, 32], BF16, kind="ExternalInput")
    WoCT_d = nc.dram_tensor("WoCT", [D, IN], BF16, kind="ExternalInput")
    WoXT_d = nc.dram_tensor("WoXT", [IN, IN], BF16, kind="ExternalInput")
    ident_d = nc.dram_tensor("ident", [128, 128], BF16, kind="ExternalInput")
    bc_d = nc.dram_tensor("bc2", [128, 2], F32, kind="ExternalInput")
    bout_d = nc.dram_tensor("bout4", [128, 4], F32, kind="ExternalInput")

    attn_d = nc.dram_tensor("attn_outT", [NJ, 128, 4, TT], F32, kind="ExternalOutput")
    align_d = nc.dram_tensor("align_out", [NJ, 128, S], F32, kind="ExternalOutput")

    with tile.TileContext(nc) as tc, ExitStack() as ctx:
        const = ctx.enter_context(tc.tile_pool(name="const", bufs=1))
        pin = ctx.enter_context(tc.tile_pool(name="pin", bufs=NJ))
        pmid = ctx.enter_context(tc.tile_pool(name="pmid", bufs=NJ))
        pz = ctx.enter_context(tc.tile_pool(name="pz", bufs=2))
        pth = ctx.enter_context(tc.tile_pool(name="pth", bufs=2))
        pep = ctx.enter_context(tc.tile_pool(name="pep", bufs=NJ))
        psA = ctx.enter_context(tc.tile_pool(name="psA", bufs=NJ, space="PSUM"))
        psU = ctx.enter_context(tc.tile_pool(name="psU", bufs=1, space="PSUM"))
        psW = ctx.enter_context(tc.tile_pool(name="psW", bufs=1, space="PSUM"))
        psT = ctx.enter_context(tc.tile_pool(name="psT", bufs=1, space="PSUM"))
        psC = ctx.enter_context(tc.tile_pool(name="psC", bufs=1, space="PSUM"))
        psO = ctx.enter_context(tc.tile_pool(name="psO", bufs=1, space="PSUM"))

        def load(pool, shape, dt, src, tag):
            t = pool.tile(shape, dt, tag=tag)
            nc.sync.dma_start(t[...], src)
            return t

        wqT = load(const, [128, 4, D], BF16,
                   WqT_d.ap().rearrange("(c p) d -> p c d", p=128), "wqT")
        wcT = load(const, [128, 2, D], BF16,
                   WcT_d.ap().rearrange("(c p) d -> p c d", p=128), "wcT")
        vcols = load(const, [128, 2, 32, 32], BF16, vcols_d.ap(), "vcols")
        woCT = load(const, [128, 2, IN], BF16,
                    WoCT_d.ap().rearrange("(c p) o -> p c o", p=128), "woCT")
        woXT = load(const, [128, 4, IN], BF16,
                    WoXT_d.ap().rearrange("(c p) o -> p c o", p=128), "woXT")
        ident = load(const, [128, 128], BF16, ident_d.ap(), "ident")
        bc2 = load(const, [128, 2], F32, bc_d.ap(), "bc2")
        bout4 = load(const, [128, 4], F32, bout_d.ap(), "bout4")

        for j in range(NJ):
            xT = load(pin, [128, 4, TT], BF16,
                      xT_d.ap()[j].rearrange("(c p) t -> p c t", p=128), "xT")
            memsT = load(pin, [128, 2, S], BF16,
                         memsT_d.ap()[j].rearrange("(c p) s -> p c s", p=128), "memsT")
            memsL = load(pin, [128, 4, D], BF16,
                         memsL_d.ap()[j].rearrange("(c p) m -> p c m", p=128), "memsL")
            maskj = load(pin, [128, S], F32, mask_d.ap()[j], "maskj")

            # wq[t,d] for this tile, laid out [d_half(128), h, t]
            wq_ps = psW.tile([128, 2 * TT], F32, tag="wq_ps")
            for h in range(2):
                for ic in range(4):
                    nc.tensor.matmul(wq_ps[:, h * TT:(h + 1) * TT],
                                     wqT[:, ic, h * 128:(h + 1) * 128],
                                     xT[:, ic, :],
                                     start=(ic == 0), stop=(ic == 3))
            wq_sb = pmid.tile([128, 2, TT], F32, tag="wq_sb")
            nc.vector.tensor_copy(wq_sb[...], wq_ps[...])

            # uh[s,d] laid out [d_half(128), h, s], + bc
            uh_sb = pmid.tile([128, 2, S], F32, tag="uh_sb")
            for h in range(2):
                uh_ps = psU.tile([128, S], F32, tag="uh_ps")
                for mc in range(2):
                    nc.tensor.matmul(uh_ps[...],
                                     wcT[:, mc, h * 128:(h + 1) * 128],
                                     memsT[:, mc, :],
                                     start=(mc == 0), stop=(mc == 1))
                nc.vector.tensor_scalar_add(uh_sb[:, h, :], uh_ps[...],
                                            bc2[:, h:h + 1])

            # main loop: align[t,s] = sum_d v_d tanh(uh + wq[t])
            align_ps = psA.tile([128, S], F32, tag="align_ps")
            for g in range(TT // G):
                z = pz.tile([128, G * 2 * S], F32, tag="z")
                for jj in range(G):
                    t = g * G + jj
                    for h in range(2):
                        nc.vector.tensor_scalar_add(
                            z[:, (2 * jj + h) * S:(2 * jj + h + 1) * S],
                            uh_sb[:, h, :], wq_sb[:, h, t:t + 1])
                th = pth.tile([128, G * 2 * S], BF16, tag="th")
                nc.scalar.activation(th[...], z[...], TANH)
                for jj in range(G):
                    t = g * G + jj
                    k, c = t // 32, t % 32
                    for h in range(2):
                        nc.tensor.matmul(
                            align_ps[32 * k:32 * (k + 1), :],
                            vcols[:, h, c, :],
                            th[:, (2 * jj + h) * S:(2 * jj + h + 1) * S],
                            start=(c == 0 and h == 0),
                            stop=(c == 31 and h == 1),
                            tile_position=(0, 32 * k))

            # softmax (no max-sub: |align| <= ~10), multiplicative mask
            av_e = pep.tile([128, S], F32, tag="av_e")
            nc.scalar.activation(av_e[...], align_ps[...], EXP)
            av_m = pep.tile([128, S], F32, tag="av_m")
            nc.vector.tensor_mul(av_m[...], av_e[...], maskj[...])
            ssum = pep.tile([128, 1], F32, tag="ssum")
            nc.vector.reduce_sum(ssum[...], av_m[...], axis=mybir.AxisListType.X)
            rcp = pep.tile([128, 1], F32, tag="rcp")
            nc.vector.reciprocal(rcp[...], ssum[...])
            av = pep.tile([128, S], F32, tag="av")
            nc.vector.tensor_scalar_mul(av[...], av_m[...], rcp[...])
            nc.sync.dma_start(align_d.ap()[j], av[...])

            av_bf = pep.tile([128, S], BF16, tag="av_bf")
            nc.vector.tensor_copy(av_bf[...], av[...])

            # transpose av -> [s, t] for the c matmul
            avT = pep.tile([128, 4, TT], BF16, tag="avT")
            for sb in range(4):
                tp = psT.tile([128, 128], BF16, tag="tp")
                nc.tensor.transpose(tp[...], av_bf[:, sb * 128:(sb + 1) * 128],
                                    ident[...])
                nc.vector.tensor_copy(avT[:, sb, :], tp[...])

            # c[t,m] laid out [m_half(128), mh, t]
            c_ps = psC.tile([128, 2 * TT], F32, tag="c_ps")
            for mh in range(2):
                for sb in range(4):
                    nc.tensor.matmul(c_ps[:, mh * TT:(mh + 1) * TT],
                                     memsL[:, sb, mh * 128:(mh + 1) * 128],
                                     avT[:, sb, :],
                                     start=(sb == 0), stop=(sb == 3))
            c_bf = pep.tile([128, 2, TT], BF16, tag="c_bf")
            nc.vector.tensor_copy(c_bf[...], c_ps[...])

            # attn[t,o] = Wout @ [c|x] + bout, laid out [o_within(128), oc, t]
            at_ps = psO.tile([128, 4 * TT], F32, tag="at_ps")
            for oc in range(4):
                for mh in range(2):
                    nc.tensor.matmul(at_ps[:, oc * TT:(oc + 1) * TT],
                                     woCT[:, mh, oc * 128:(oc + 1) * 128],
                                     c_bf[:, mh, :],
                                     start=(mh == 0), stop=False)
                for ic in range(4):
                    nc.tensor.matmul(at_ps[:, oc * TT:(oc + 1) * TT],
                                     woXT[:, ic, oc * 128:(oc + 1) * 128],
                                     xT[:, ic, :],
                                     start=False, stop=(ic == 3))
            attn_sb = pep.tile([128, 4, TT], F32, tag="attn_sb")
            for oc in range(4):
                nc.vector.tensor_scalar_add(attn_sb[:, oc, :],
                                            at_ps[:, oc * TT:(oc + 1) * TT],
                                            bout4[:, oc:oc + 1])
            nc.sync.dma_start(attn_d.ap()[j], attn_sb[...])

    nc.compile()
    return nc


def _prep_inputs(inputs, mems, mem_masks, Wq, Wc, bc, v, Wout, bout):
    x = np.ascontiguousarray(np.asarray(inputs, dtype=np.float32))
    mems = np.ascontiguousarray(np.asarray(mems, dtype=np.float32))
    L = np.asarray(mem_masks).astype(np.int64)
    Wq = np.asarray(Wq, dtype=np.float32)
    Wc = np.asarray(Wc, dtype=np.float32)
    bc = np.asarray(bc, dtype=np.float32)
    v = np.asarray(v, dtype=np.float32)
    Wout = np.asarray(Wout, dtype=np.float32)
    bout = np.asarray(bout, dtype=np.float32)

    WqT = np.ascontiguousarray(Wq.T).astype(BF)               # [IN, D]
    WcT = np.ascontiguousarray(Wc.T).astype(BF)               # [D, D]
    WoCT = np.ascontiguousarray(Wout[:, :D].T).astype(BF)     # [D, IN]
    WoXT = np.ascontiguousarray(Wout[:, D:].T).astype(BF)     # [IN, IN]
    ident = np.eye(128, dtype=np.float32).astype(BF)
    bc2 = np.ascontiguousarray(bc.reshape(2, 128).T).astype(np.float32)
    bout4 = np.ascontiguousarray(bout.reshape(4, 128).T).astype(np.float32)
    vcols = np.zeros((128, 2, 32, 32), np.float32)
    for h in range(2):
        for c in range(32):
            vcols[:, h, c, c] = v[h * 128:(h + 1) * 128]
    vcols = vcols.astype(BF)

    shared = dict(WqT=WqT, WcT=WcT, vcols=vcols, WoCT=WoCT, WoXT=WoXT,
                  ident=ident, bc2=bc2, bout4=bout4)

    tiles = [(b, tt) for b in range(B) for tt in range(T // TT)]
    in_maps = []
    for core in range(NC):
        xT = np.zeros((NJ, IN, TT), np.float32)
        memsT = np.zeros((NJ, D, S), np.float32)
        memsL = np.zeros((NJ, S, D), np.float32)
        mask = np.zeros((NJ, 128, S), np.float32)
        for j in range(NJ):
            b, tt = tiles[core * NJ + j]
            xT[j] = x[b, tt * TT:(tt + 1) * TT, :].T
            memsT[j] = mems[b].T
            memsL[j] = mems[b]
            mask[j, :, :] = (np.arange(S) < int(L[b])).astype(np.float32)[None, :]
        m = dict(shared)
        m["xT"] = np.ascontiguousarray(xT).astype(BF)
        m["memsT"] = np.ascontiguousarray(memsT).astype(BF)
        m["memsL"] = np.ascontiguousarray(memsL).astype(BF)
        m["mask"] = np.ascontiguousarray(mask)
        in_maps.append(m)
    return in_maps, tiles


def kernel(**inputs):
    global _BUILT, LAST_RESULT
    in_maps, tiles = _prep_inputs(**inputs)
    if _BUILT is None:
        _BUILT = _build()
    res = run_bass_kernel_spmd(_BUILT, in_maps, core_ids=list(range(NC)))
    LAST_RESULT = res

    attn_h = np.zeros((B, T, IN), np.float32)
    align_v = np.zeros((B, T, S), np.float32)
    for core in range(NC):
        for j in range(NJ):
            b, tt = tiles[core * NJ + j]
            at = res.results[core]["attn_outT"][j]        # [128(p), 4(oc), 128(t)]
            attn_h[b, tt * TT:(tt + 1) * TT, :] = \
                np.transpose(at, (2, 1, 0)).reshape(TT, IN)
            align_v[b, tt * TT:(tt + 1) * TT, :] = res.results[core]["align_out"][j]
    return attn_h, align_v


# revision 6
# speedup vs baseline: 1.0285x; 1.0285x over previous
"""Bahdanau additive attention on 8 TRN2 NeuronCores (Bass/Tile).

Reference computation (B=4, T=512, S=512, D=256, IN=512):
    wq[b,t,d]   = sum_i x[b,t,i]   * Wq[d,i]
    uh[b,s,d]   = sum_m mems[b,s,m]* Wc[d,m] + bc[d]
    align[b,t,s]= sum_d v[d] * tanh(wq[b,t,d] + uh[b,s,d])     (masked s>=L_b -> -inf)
    av          = softmax_s(align)
    c[b,t,m]    = sum_s av[b,t,s] * mems[b,s,m]
    attn[b,t,o] = sum_k [c|x][b,t,k] * Wout[o,k] + bout[o]
    returns (attn, av)

Sharding: 16 (batch, t-tile-of-128) blocks, 2 per core (pure data parallel,
no collectives).  Per (t, d-half): DVE broadcast-add z = uh + wq[t] (fp32),
ACT tanh batched over 8 such slices (one big ACTIVATE), PE reduces over the
d-partition dim with a 32-column one-hot v weight into the PSUM row for t.
Softmax is exp -> multiplicative mask -> sum -> reciprocal scale (no max
subtraction needed: |align| <= sum|v| ~ 10).  All matmul inputs bf16,
accumulation fp32.
"""
import numpy as np
import ml_dtypes
from contextlib import ExitStack

import concourse.bass as bass
import concourse.bacc as bacc
import concourse.mybir as mybir
import concourse.tile as tile
from concourse.bass_utils import run_bass_kernel_spmd

F32 = mybir.dt.float32
BF16 = mybir.dt.bfloat16
TANH = mybir.ActivationFunctionType.Tanh
EXP = mybir.ActivationFunctionType.Exp
BF = ml_dtypes.bfloat16

B, T, S, D, IN = 4, 512, 512, 256, 512
NC = 8           # cores
NJ = 2           # t-tiles per core
TT = 128         # t rows per tile
G = 8            # t's per ACT batch (free = G*2*S = 8192)

_BUILT = None
LAST_RESULT = None


def _build():
    nc = bacc.Bacc("TRN2", target_bir_lowering=False, debug=False,
                   enable_asserts=False, num_devices=NC)

    xT_d = nc.dram_tensor("xT", [NJ, IN, TT], BF16, kind="ExternalInput")
    memsT_d = nc.dram_tensor("memsT", [NJ, D, S], BF16, kind="ExternalInput")
    memsL_d = nc.dram_tensor("memsL", [NJ, S, D], BF16, kind="ExternalInput")
    mask_d = nc.dram_tensor("mask", [NJ, 128, S], F32, kind="ExternalInput")
    WqT_d = nc.dram_tensor("WqT", [IN, D], BF16, kind="ExternalInput")
    WcT_d = nc.dram_tensor("WcT", [D, D], BF16, kind="ExternalInput")
    vcols_d = nc.dram_tensor("vcols", [128, 2, 32, 32], BF16, kind="ExternalInput")
    WoCT_d = nc.dram_tensor("WoCT", [D, IN], BF16, kind="ExternalInput")
    WoXT_d = nc.dram_tensor("WoXT", [IN, IN], BF16, kind="ExternalInput")
    ident_d = nc.dram_tensor("ident", [128, 128], BF16, kind="ExternalInput")
    bc_d = nc.dram_tensor("bc2", [128, 2], F32, kind="ExternalInput")
    bout_d = nc.dram_tensor("bout4", [128, 4], F32, kind="ExternalInput")

    attn_d = nc.dram_tensor("attn_outT", [NJ, 128, 4, TT], F32, kind="ExternalOutput")
    align_d = nc.dram_tensor("align_out", [NJ, 128, S], F32, kind="ExternalOutput")

    with tile.TileContext(nc) as tc, ExitStack() as ctx:
        const = ctx.enter_context(tc.tile_pool(name="const", bufs=1))
        pin = ctx.enter_context(tc.tile_pool(name="pin", bufs=NJ))
        pmid = ctx.enter_context(tc.tile_pool(name="pmid", bufs=NJ))
        pz = ctx.enter_context(tc.tile_pool(name="pz", bufs=2))
        pth = ctx.enter_context(tc.tile_pool(name="pth", bufs=2))
        pep = ctx.enter_context(tc.tile_pool(name="pep", bufs=NJ))
        psA = ctx.enter_context(tc.tile_pool(name="psA", bufs=NJ, space="PSUM"))
        psU = ctx.enter_context(tc.tile_pool(name="psU", bufs=1, space="PSUM"))
        psW = ctx.enter_context(tc.tile_pool(name="psW", bufs=1, space="PSUM"))
        psT = ctx.enter_context(tc.tile_pool(name="psT", bufs=1, space="PSUM"))
        psC = ctx.enter_context(tc.tile_pool(name="psC", bufs=1, space="PSUM"))
        psO = ctx.enter_context(tc.tile_pool(name="psO", bufs=1, space="PSUM"))

        def load(pool, shape, dt, src, tag):
            t = pool.tile(shape, dt, tag=tag)
            nc.sync.dma_start(t[...], src)
            return t

        wqT = load(const, [128, 4, D], BF16,
                   WqT_d.ap().rearrange("(c p) d -> p c d", p=128), "wqT")
        wcT = load(const, [128, 2, D], BF16,
                   WcT_d.ap().rearrange("(c p) d -> p c d", p=128), "wcT")
        vcols = load(const, [128, 2, 32, 32], BF16, vcols_d.ap(), "vcols")
        woCT = load(const, [128, 2, IN], BF16,
                    WoCT_d.ap().rearrange("(c p) o -> p c o", p=128), "woCT")
        woXT = load(const, [128, 4, IN], BF16,
                    WoXT_d.ap().rearrange("(c p) o -> p c o", p=128), "woXT")
        ident = load(const, [128, 128], BF16, ident_d.ap(), "ident")
        bc2 = load(const, [128, 2], F32, bc_d.ap(), "bc2")
        bout4 = load(const, [128, 4], F32, bout_d.ap(), "bout4")

        for j in range(NJ):
            xT = load(pin, [128, 4, TT], BF16,
                      xT_d.ap()[j].rearrange("(c p) t -> p c t", p=128), "xT")
            memsT = load(pin, [128, 2, S], BF16,
                         memsT_d.ap()[j].rearrange("(c p) s -> p c s", p=128), "memsT")
            memsL = load(pin, [128, 4, D], BF16,
                         memsL_d.ap()[j].rearrange("(c p) m -> p c m", p=128), "memsL")
            maskj = load(pin, [128, S], F32, mask_d.ap()[j], "maskj")

            # wq[t,d] for this tile, laid out [d_half(128), h, t]
            wq_ps = psW.tile([128, 2 * TT], F32, tag="wq_ps")
            for h in range(2):
                for ic in range(4):
                    nc.tensor.matmul(wq_ps[:, h * TT:(h + 1) * TT],
                                     wqT[:, ic, h * 128:(h + 1) * 128],
                                     xT[:, ic, :],
                                     start=(ic == 0), stop=(ic == 3))
            wq_sb = pmid.tile([128, 2, TT], F32, tag="wq_sb")
            nc.vector.tensor_copy(wq_sb[...], wq_ps[...])

            # uh[s,d] laid out [d_half(128), h, s], + bc
            uh_sb = pmid.tile([128, 2, S], BF16, tag="uh_sb")
            for h in range(2):
                uh_ps = psU.tile([128, S], F32, tag="uh_ps")
                for mc in range(2):
                    nc.tensor.matmul(uh_ps[...],
                                     wcT[:, mc, h * 128:(h + 1) * 128],
                                     memsT[:, mc, :],
                                     start=(mc == 0), stop=(mc == 1))
                nc.vector.tensor_scalar_add(uh_sb[:, h, :], uh_ps[...],
                                            bc2[:, h:h + 1])

            # main loop: align[t,s] = sum_d v_d tanh(uh + wq[t])
            align_ps = psA.tile([128, S], F32, tag="align_ps")
            for g in range(TT // G):
                z = pz.tile([128, G * 2 * S], BF16, tag="z")
                for jj in range(G):
                    t = g * G + jj
                    for h in range(2):
                        nc.vector.tensor_scalar_add(
                            z[:, (2 * jj + h) * S:(2 * jj + h + 1) * S],
                            uh_sb[:, h, :], wq_sb[:, h, t:t + 1])
                th = pth.tile([128, G * 2 * S], BF16, tag="th")
                nc.scalar.activation(th[...], z[...], TANH)
                for jj in range(G):
                    t = g * G + jj
                    k, c = t // 32, t % 32
                    for h in range(2):
                        nc.tensor.matmul(
                            align_ps[32 * k:32 * (k + 1), :],
                            vcols[:, h, c, :],
                            th[:, (2 * jj + h) * S:(2 * jj + h + 1) * S],
                            start=(c == 0 and h == 0),
                            stop=(c == 31 and h == 1),
                            tile_position=(0, 32 * k))

            # softmax (no max-sub: |align| <= ~10), multiplicative mask
            av_e = pep.tile([128, S], F32, tag="av_e")
            nc.scalar.activation(av_e[...], align_ps[...], EXP)
            av_m = pep.tile([128, S], F32, tag="av_m")
            nc.vector.tensor_mul(av_m[...], av_e[...], maskj[...])
            ssum = pep.tile([128, 1], F32, tag="ssum")
            nc.vector.reduce_sum(ssum[...], av_m[...], axis=mybir.AxisListType.X)
            rcp = pep.tile([128, 1], F32, tag="rcp")
            nc.vector.reciprocal(rcp[...], ssum[...])
            av = pep.tile([128, S], F32, tag="av")
            nc.vector.tensor_scalar_mul(av[...], av_m[...], rcp[...])
            nc.sync.dma_start(align_d.ap()[j], av[...])

            av_bf = pep.tile([128, S], BF16, tag="av_bf")
            nc.vector.tensor_copy(av_bf[...], av[...])

            # transpose av -> [s, t] for the c matmul
            avT = pep.tile([128, 4, TT], BF16, tag="avT")
            for sb in range(4):
                tp = psT.tile([128, 128], BF16, tag="tp")
                nc.tensor.transpose(tp[...], av_bf[:, sb * 128:(sb + 1) * 128],
                                    ident[...])
                nc.vector.tensor_copy(avT[:, sb, :], tp[...])

            # c[t,m] laid out [m_half(128), mh, t]
            c_ps = psC.tile([128, 2 * TT], F32, tag="c_ps")
            for mh in range(2):
                for sb in range(4):
                    nc.tensor.matmul(c_ps[:, mh * TT:(mh + 1) * TT],
                                     memsL[:, sb, mh * 128:(mh + 1) * 128],
                                     avT[:, sb, :],
                                     start=(sb == 0), stop=(sb == 3))
            c_bf = pep.tile([128, 2, TT], BF16, tag="c_bf")
            nc.vector.tensor_copy(c_bf[...], c_ps[...])

            # attn[t,o] = Wout @ [c|x] + bout, laid out [o_within(128), oc, t]
            at_ps = psO.tile([128, 4 * TT], F32, tag="at_ps")
            for oc in range(4):
                for mh in range(2):
                    nc.tensor.matmul(at_ps[:, oc * TT:(oc + 1) * TT],
                                     woCT[:, mh, oc * 128:(oc + 1) * 128],
                                     c_bf[:, mh, :],
                                     start=(mh == 0), stop=False)
                for ic in range(4):
                    nc.tensor.matmul(at_ps[:, oc * TT:(oc + 1) * TT],
                                     woXT[:, ic, oc * 128:(oc + 1) * 128],
                                     xT[:, ic, :],
                                     start=False, stop=(ic == 3))
            attn_sb = pep.tile([128, 4, TT], F32, tag="attn_sb")
            for oc in range(4):
                nc.vector.tensor_scalar_add(attn_sb[:, oc, :],
                                            at_ps[:, oc * TT:(oc + 1) * TT],
                                            bout4[:, oc:oc + 1])
            nc.sync.dma_start(attn_d.ap()[j], attn_sb[...])

    nc.compile()
    return nc


def _prep_inputs(inputs, mems, mem_masks, Wq, Wc, bc, v, Wout, bout):
    x = np.ascontiguousarray(np.asarray(inputs, dtype=np.float32))
    mems = np.ascontiguousarray(np.asarray(mems, dtype=np.float32))
    L = np.asarray(mem_masks).astype(np.int64)
    Wq = np.asarray(Wq, dtype=np.float32)
    Wc = np.asarray(Wc, dtype=np.float32)
    bc = np.asarray(bc, dtype=np.float32)
    v = np.asarray(v, dtype=np.float32)
    Wout = np.asarray(Wout, dtype=np.float32)
    bout = np.asarray(bout, dtype=np.float32)

    WqT = np.ascontiguousarray(Wq.T).astype(BF)               # [IN, D]
    WcT = np.ascontiguousarray(Wc.T).astype(BF)               # [D, D]
    WoCT = np.ascontiguousarray(Wout[:, :D].T).astype(BF)     # [D, IN]
    WoXT = np.ascontiguousarray(Wout[:, D:].T).astype(BF)     # [IN, IN]
    ident = np.eye(128, dtype=np.float32).astype(BF)
    bc2 = np.ascontiguousarray(bc.reshape(2, 128).T).astype(np.float32)
    bout4 = np.ascontiguousarray(bout.reshape(4, 128).T).astype(np.float32)
    vcols = np.zeros((128, 2, 32, 32), np.float32)
    for h in range(2):
        for c in range(32):
            vcols[:, h, c, c] = v[h * 128:(h + 1) * 128]
    vcols = vcols.astype(BF)

    shared = dict(WqT=WqT, WcT=WcT, vcols=vcols, WoCT=WoCT, WoXT=WoXT,
                  ident=ident, bc2=bc2, bout4=bout4)

    tiles = [(b, tt) for b in range(B) for tt in range(T // TT)]
    in_maps = []
    for core in range(NC):
        xT = np.zeros((NJ, IN, TT), np.float32)
        memsT = np.zeros((NJ, D, S), np.float32)
        memsL = np.zeros((NJ, S, D), np.float32)
        mask = np.zeros((NJ, 128, S), np.float32)
        for j in range(NJ):
            b, tt = tiles[core * NJ + j]
            xT[j] = x[b, tt * TT:(tt + 1) * TT, :].T
            memsT[j] = mems[b].T
            memsL[j] = mems[b]
            mask[j, :, :] = (np.arange(S) < int(L[b])).astype(np.float32)[None, :]
        m = dict(shared)
        m["xT"] = np.ascontiguousarray(xT).astype(BF)
        m["memsT"] = np.ascontiguousarray(memsT).astype(BF)
        m["memsL"] = np.ascontiguousarray(memsL).astype(BF)
        m["mask"] = np.ascontiguousarray(mask)
        in_maps.append(m)
    return in_maps, tiles


def kernel(**inputs):
    global _BUILT, LAST_RESULT
    in_maps, tiles = _prep_inputs(**inputs)
    if _BUILT is None:
        _BUILT = _build()
    res = run_bass_kernel_spmd(_BUILT, in_maps, core_ids=list(range(NC)))
    LAST_RESULT = res

    attn_h = np.zeros((B, T, IN), np.float32)
    align_v = np.zeros((B, T, S), np.float32)
    for core in range(NC):
        for j in range(NJ):
            b, tt = tiles[core * NJ + j]
            at = res.results[core]["attn_outT"][j]        # [128(p), 4(oc), 128(t)]
            attn_h[b, tt * TT:(tt + 1) * TT, :] = \
                np.transpose(at, (2, 1, 0)).reshape(TT, IN)
            align_v[b, tt * TT:(tt + 1) * TT, :] = res.results[core]["align_out"][j]
    return attn_h, align_v
